# revision 1
# baseline (speedup 1.0000x reference)
"""Dual (global + local-masked) BERT self-attention on 8 Trainium2 NeuronCores.

Problem: B=2, S=2048, H=1024, NH=16 heads of DH=64.
  q/k/v = hidden @ W{q,k,v}.T + b ; scores = q k^T / 8
  probs_g = softmax(scores + attention_mask)         (additive, zeros in spec)
  probs_l = softmax(scores + (-inf where local_mask==0))
  out     = gate * (probs_l @ v) + (1-gate) * (probs_g @ v)

Sharding: 32 (batch, head) pairs -> 4 heads per core (core c: batch c//4,
heads 4*(c%4)..+4). Each core computes its heads' projections + dual
attention independently; no collectives.

Per-core kernel (all layouts transposed so softmax reductions ride the
TensorEngine):
  - X^T, W slices, local mask (as 0/1 bf16, transposed) are DMA'd in.
  - Q^T,K^T [128 dims (2-head pair), S] and natural-layout V (+ ones
    column) are computed on PE in bf16 (f32 PSUM accumulation).
  - per (head, 1024-query chunk, 128-key tile): scores^T [128 keys, 1024 q]
    on PE (K=64 contraction), e = exp(scores) once on ACT (shared by both
    branches, bf16 out), e_l = e * mask on DVE.
  - ctx^T [65, 1024] accumulated over key tiles: lhsT = [V_tile | ones], so
    PSUM row 64 accumulates the softmax denominator for free.
  - epilogue: sum rows -> base-0 tile (ACT copy + tiny DMA),
    reciprocal_approx * gate rows -> per-q coefficients (DVE), broadcast to
    64 partitions via K=2 selector matmuls, combine on DVE, DMA out.
No max-subtraction in softmax: scores are O(+-5), exp is safe in fp32 and
softmax is shift-invariant, so the reference is matched to ~4e-3.
Output per core: [256 dims, 2048 q] f32; host transposes/reassembles.
Epilogue releases ctx PSUM early (copy to SBUF right after the k-loop)
so the next head's accumulation overlaps the normalize/gate chain.
Measured ~270-300 us/core on TRN2 (repeat-slope timing; dispatch excluded).
"""

import sys

sys.path.insert(0, "/opt/trn_rl_repo")

import numpy as np
import ml_dtypes

B, S, H, NH, DH = 2, 2048, 1024, 16, 64
NCORES = 8
HPC = 4          # heads per core
MPC = HPC // 2   # head pairs per core
QC = 1024        # query chunk (free dim of scores/ctx psums)
NQC = S // QC
KT = S // 128    # key tiles
XT_T = H // 128  # X^T k-tiles for projections

_BUILT = {}


def _build(use_em: bool, repeat: int = 1, pairs: bool = False, abl: str = "", qcw: int = QC, ctxbufs: int = 1, scbufs: int = 2, tune: bool = True, has_b: bool = False, gbc: bool = False):
    from contextlib import ExitStack

    import concourse.mybir as mybir
    from concourse import bacc, tile

    f32 = mybir.dt.float32
    bf16 = mybir.dt.bfloat16
    AF = mybir.ActivationFunctionType

    nc = bacc.Bacc("TRN2", target_bir_lowering=False, debug=False)

    xt_d = nc.dram_tensor("xt", [H, S], bf16, kind="ExternalInput").ap()
    wq_d = nc.dram_tensor("wq", [H, 256], bf16, kind="ExternalInput").ap()
    wk_d = nc.dram_tensor("wk", [H, 256], bf16, kind="ExternalInput").ap()
    wv_d = nc.dram_tensor("wv", [H, 256], bf16, kind="ExternalInput").ap()
    bqk_d = nc.dram_tensor("bqk", [2, 256], f32, kind="ExternalInput").ap()
    bv_d = nc.dram_tensor("bv", [1, 256], bf16, kind="ExternalInput").ap()
    msk_d = nc.dram_tensor("msk", [KT, 128, S], bf16, kind="ExternalInput").ap()
    # gt[r, h, q]: head h, r = (gate_h, 1-gate_h)
    gt_d = nc.dram_tensor("gt", [2, HPC, S], f32, kind="ExternalInput").ap()
    # sel[r, j, d] = 1.0 if r == j else 0 — K=2 broadcast selectors
    sel_d = nc.dram_tensor("sel", [4, 4, 64], bf16, kind="ExternalInput").ap()
    if use_em:
        em_d = nc.dram_tensor("em", [KT, 128], f32, kind="ExternalInput").ap()
    out_d = nc.dram_tensor("out", [HPC * DH, S], f32, kind="ExternalOutput").ap()

    with tile.TileContext(nc) as tc, ExitStack() as ctx:
        big = ctx.enter_context(tc.tile_pool(name="big", bufs=1))

        xt_sb = big.tile([128, XT_T, S], bf16, name="xt_sb")
        for t in range(XT_T):
            nc.sync.dma_start(xt_sb[:, t, :], xt_d[t * 128:(t + 1) * 128, :])
        w_sbs = {}
        for nm, d in (("wq", wq_d), ("wk", wk_d), ("wv", wv_d)):
            w_sb = big.tile([128, XT_T, 256], bf16, name=f"{nm}_sb")
            for t in range(XT_T):
                nc.sync.dma_start(w_sb[:, t, :], d[t * 128:(t + 1) * 128, :])
            w_sbs[nm] = w_sb
        msk_sb = big.tile([128, KT, S], bf16, name="msk_sb")
        for t in range(KT):
            nc.sync.dma_start(msk_sb[:, t, :], msk_d[t])
        sel_sb = big.tile([4, 4, 64], bf16, name="sel_sb")
        nc.sync.dma_start(sel_sb, sel_d)
        bqk_sb = big.tile([128, 2, 2], f32, name="bqk_sb")
        nc.sync.dma_start(
            bqk_sb, bqk_d.rearrange("c (t p) -> p c t", p=128)
        )
        bv_sb = big.tile([1, 256], bf16, name="bv_sb")
        nc.sync.dma_start(bv_sb, bv_d)
        if use_em:
            em_sb = big.tile([128, KT], f32, name="em_sb")
            nc.sync.dma_start(em_sb, em_d.rearrange("t p -> p t"))

        ones_r = big.tile([1, 128], bf16, name="ones_r")
        nc.vector.memset(ones_r, 1.0)

        qt_sb = big.tile([128, MPC, S], bf16, name="qt_sb")
        kt_sb = big.tile([128, MPC, S], bf16, name="kt_sb")
        v_sb = big.tile([128, KT, HPC, 65], bf16, name="v_sb")
        nc.vector.memset(v_sb[:, :, :, 64:65], 1.0)

        for _rep in range(repeat):
            # ---- projections: Q^T, K^T (transposed), V (natural) ----
            with tc.tile_pool(name="pproj", bufs=2, space="PSUM") as pproj:
                for m in range(MPC):
                    for ci, (wn, dst) in enumerate((("wq", qt_sb), ("wk", kt_sb))):
                        w_sb = w_sbs[wn]
                        for nq in range(S // 1024):
                            ps = pproj.tile([128, 1024], f32, tag="pp")
                            for t in range(XT_T):
                                for hlf in range(2):
                                    nc.tensor.matmul(
                                        ps[:, hlf * 512:(hlf + 1) * 512],
                                        lhsT=w_sb[:, t, m * 128:(m + 1) * 128],
                                        rhs=xt_sb[:, t, nq * 1024 + hlf * 512:
                                                  nq * 1024 + (hlf + 1) * 512],
                                        start=(t == 0),
                                        stop=(t == XT_T - 1),
                                    )
                            nc.scalar.activation(
                                dst[:, m, nq * 1024:(nq + 1) * 1024], ps,
                                AF.Identity, bias=bqk_sb[:, ci, m:m + 1], scale=1.0,
                            )
            with tc.tile_pool(name="pv", bufs=2, space="PSUM") as pv:
                for st in range(KT):
                    ps = pv.tile([128, 256], f32, tag="pv")
                    for t in range(XT_T):
                        nc.tensor.matmul(
                            ps,
                            lhsT=xt_sb[:, t, st * 128:(st + 1) * 128],
                            rhs=w_sbs["wv"][:, t, :],
                            start=(t == 0),
                            stop=(t == XT_T - 1 and not has_b),
                        )
                    if has_b:
                        nc.tensor.matmul(
                            ps, lhsT=ones_r, rhs=bv_sb, start=False, stop=True
                        )
                    nc.scalar.activation(
                        v_sb[:, st, :, 0:64],
                        ps.rearrange("p (h d) -> p h d", h=HPC),
                        AF.Copy,
                    )

            # ---- dual attention ----
            att_ctx = ExitStack()
            nbuf = 4 if pairs else scbufs
            psc = att_ctx.enter_context(tc.tile_pool(name="psc", bufs=nbuf, space="PSUM"))
            pctx = att_ctx.enter_context(tc.tile_pool(name="pctx", bufs=ctxbufs, space="PSUM"))
            pe = att_ctx.enter_context(tc.tile_pool(name="pe", bufs=8 if tune else 6))
            pt = att_ctx.enter_context(tc.tile_pool(name="pt", bufs=2))
            po = att_ctx.enter_context(tc.tile_pool(name="po", bufs=2))
            pc = att_ctx.enter_context(tc.tile_pool(name="pc", bufs=8))

            if pairs:
                Q2 = 512
                for m in range(MPC):
                    hA, hB = 2 * m, 2 * m + 1
                    for qc in range(S // Q2):
                        qs = slice(qc * Q2, (qc + 1) * Q2)
                        cps = [pctx.tile([65, Q2], f32, name=f"c{j}", tag=f"c{j}")
                               for j in range(4)]  # (Ag, Al, Bg, Bl)
                        for t in range(KT):
                            st0, st1 = (t == 0), (t == KT - 1)
                            psA = psc.tile([128, Q2], f32, name="psA", tag="sc")
                            psB = psc.tile([128, Q2], f32, name="psB", tag="sc")
                            nc.tensor.matmul(
                                psA, lhsT=kt_sb[0:64, m, t * 128:(t + 1) * 128],
                                rhs=qt_sb[0:64, m, qs], start=True, stop=True)
                            nc.tensor.matmul(
                                psB, lhsT=kt_sb[64:128, m, t * 128:(t + 1) * 128],
                                rhs=qt_sb[64:128, m, qs], start=True, stop=True)
                            eA = pe.tile([128, Q2], bf16, name="eA", tag="e")
                            eB = pe.tile([128, Q2], bf16, name="eB", tag="e")
                            nc.scalar.activation(eA, psA, AF.Exp)
                            nc.scalar.activation(eB, psB, AF.Exp)
                            elA = pe.tile([128, Q2], bf16, name="elA", tag="e")
                            elB = pe.tile([128, Q2], bf16, name="elB", tag="e")
                            nc.vector.tensor_mul(elA, eA, msk_sb[:, t, qs])
                            nc.vector.tensor_mul(elB, eB, msk_sb[:, t, qs])
                            if use_em:
                                egA = pe.tile([128, Q2], bf16, name="egA", tag="e")
                                egB = pe.tile([128, Q2], bf16, name="egB", tag="e")
                                nc.vector.tensor_scalar_mul(egA, eA, em_sb[:, t:t + 1])
                                nc.vector.tensor_scalar_mul(egB, eB, em_sb[:, t:t + 1])
                            else:
                                egA, egB = eA, eB
                            for j, ee in ((0, egA), (1, elA), (2, egB), (3, elB)):
                                nc.tensor.matmul(
                                    cps[j], lhsT=v_sb[:, t, 2 * m + j // 2, :],
                                    rhs=ee, start=st0, stop=st1)
                        # epilogue (rows: l_A, g_A, l_B, g_B)
                        stage = pc.tile([65, 4, Q2], f32, name="stage",
                                        tag="stage", bufs=2)
                        for j, src in enumerate((cps[1], cps[0], cps[3], cps[2])):
                            nc.scalar.activation(stage[64:65, j, :],
                                                 src[64:65, :], AF.Copy)
                        sums4 = pc.tile([4, Q2], f32, name="sums4", tag="sums", bufs=2)
                        nc.sync.dma_start(sums4, stage[64:65, :, :])
                        rec4 = pc.tile([4, Q2], f32, name="rec4", tag="sums", bufs=2)
                        nc.vector.reciprocal_approx_fast(rec4, sums4)
                        gtt4 = pc.tile([4, Q2], f32, name="gtt4", tag="gtt", bufs=2)
                        nc.sync.dma_start(gtt4[0:2, :], gt_d[:, hA, qs])
                        nc.sync.dma_start(gtt4[2:4, :], gt_d[:, hB, qs])
                        coef4 = pc.tile([4, Q2], bf16, name="coef4",
                                        tag="coefb", bufs=2)
                        nc.vector.tensor_mul(coef4, rec4, gtt4)
                        for jj, hh in ((0, hA), (1, hB)):
                            ctg2, ctl2 = cps[2 * jj], cps[2 * jj + 1]
                            bcl = psc.tile([64, Q2], f32, name="bcl", tag="sc")
                            bcg = psc.tile([64, Q2], f32, name="bcg", tag="sc")
                            nc.tensor.matmul(bcl, lhsT=sel_sb[:, 2 * jj, :],
                                             rhs=coef4, start=True, stop=True)
                            nc.tensor.matmul(bcg, lhsT=sel_sb[:, 2 * jj + 1, :],
                                             rhs=coef4, start=True, stop=True)
                            bcl_s = pt.tile([64, Q2], f32, name="bcl_s", tag="bc")
                            bcg_s = pt.tile([64, Q2], f32, name="bcg_s", tag="bc")
                            nc.scalar.activation(bcl_s, bcl, AF.Copy)
                            nc.scalar.activation(bcg_s, bcg, AF.Copy)
                            t1 = pt.tile([64, Q2], f32, name="t1", tag="t")
                            t2 = pt.tile([64, Q2], f32, name="t2", tag="t")
                            nc.vector.tensor_mul(t1, ctl2[0:64, :], bcl_s)
                            nc.vector.tensor_mul(t2, ctg2[0:64, :], bcg_s)
                            o = po.tile([64, Q2], f32, name="o", tag="o")
                            nc.vector.tensor_add(o, t1, t2)
                            nc.sync.dma_start(out_d[hh * 64:(hh + 1) * 64, qs], o)
                att_ctx.close()
                continue

            for h in range(HPC):
                m, par = h // 2, h % 2
                ksl = slice(64 * par, 64 * par + 64)  # head's dims within the pair
                for qc in range(S // qcw):
                    qs = slice(qc * qcw, (qc + 1) * qcw)
                    ctg = pctx.tile([65, qcw], f32, name="ctg", tag="ctxg")
                    ctl = pctx.tile([65, qcw], f32, name="ctl", tag="ctxl")
                    for t in range(KT):
                        st0 = (t == 0)
                        st1 = (t == KT - 1)
                        ps = psc.tile([128, qcw], f32, name="ps", tag="sc")
                        for hlf in range(qcw // 512):
                            nc.tensor.matmul(
                                ps[:, hlf * 512:(hlf + 1) * 512],
                                lhsT=kt_sb[ksl, m, t * 128:(t + 1) * 128],
                                rhs=qt_sb[ksl, m, qc * qcw + hlf * 512:
                                          qc * qcw + (hlf + 1) * 512],
                                start=True, stop=True,
                            )
                        e = pe.tile([128, qcw], bf16, name="e", tag="e")
                        nc.scalar.activation(e, ps, AF.Exp)
                        el = e
                        if abl != "noloc":
                            el = pe.tile([128, qcw], bf16, name="el", tag="e")
                            if tune:
                                for hlf in range(qcw // 512):
                                    h5 = slice(hlf * 512, (hlf + 1) * 512)
                                    nc.vector.tensor_mul(el[:, h5], e[:, h5],
                                                         msk_sb[:, t, qc * qcw + hlf * 512:
                                                                qc * qcw + (hlf + 1) * 512])
                            else:
                                nc.vector.tensor_mul(el, e, msk_sb[:, t, qs])
                        if use_em:
                            eg = pe.tile([128, qcw], bf16, name="eg", tag="e")
                            nc.vector.tensor_scalar_mul(eg, e, em_sb[:, t:t + 1])
                        else:
                            eg = e
                        if abl == "noctx":
                            continue
                        for hlf in range(qcw // 512):
                            h5 = slice(hlf * 512, (hlf + 1) * 512)
                            nc.tensor.matmul(ctg[:, h5], lhsT=v_sb[:, t, h, :],
                                             rhs=eg[:, h5], start=st0, stop=st1)
                            if abl != "noloc":
                                nc.tensor.matmul(ctl[:, h5], lhsT=v_sb[:, t, h, :],
                                                 rhs=el[:, h5], start=st0, stop=st1)
                    if abl:
                        o = po.tile([64, qcw], f32, name="o", tag="o")
                        src_abl = el[0:64, :] if abl == "noctx" else ctg[0:64, :]
                        nc.scalar.activation(o, src_abl, AF.Copy)
                        nc.sync.dma_start(out_d[h * 64:(h + 1) * 64, qs], o)
                        continue
                    if tune:
                        # release ctx PSUM early: copy both ctx tiles to SBUF, then run
                        # the whole epilogue from SBUF while the next head accumulates.
                        ctl_s = pt.tile([65, qcw], f32, name="ctl_s", tag="cts")
                        ctg_s = pt.tile([65, qcw], f32, name="ctg_s", tag="cts")
                        nc.scalar.activation(ctl_s, ctl, AF.Copy)
                        nc.scalar.activation(ctg_s, ctg, AF.Copy)
                        sums2 = pc.tile([2, qcw], f32, name="sums2", tag="sums", bufs=2)
                        nc.sync.dma_start(sums2[0:1, :], ctl_s[64:65, :])
                        nc.sync.dma_start(sums2[1:2, :], ctg_s[64:65, :])
                        rec2 = pc.tile([2, qcw], f32, name="rec2", tag="sums", bufs=2)
                        nc.vector.reciprocal_approx_fast(rec2, sums2)
                        gtt = pc.tile([2, qcw], f32, name="gtt", tag="gtt", bufs=2)
                        nc.sync.dma_start(gtt, gt_d[:, h, qs])
                        if gbc:
                            sl1 = pc.tile([1, qcw], f32, name="sl1", tag="s1", bufs=2)
                            sg1 = pc.tile([1, qcw], f32, name="sg1", tag="s1", bufs=2)
                            nc.sync.dma_start(sl1, ctl_s[64:65, :])
                            nc.sync.dma_start(sg1, ctg_s[64:65, :])
                            rl1 = pc.tile([1, qcw], f32, name="rl1", tag="s1", bufs=2)
                            rg1 = pc.tile([1, qcw], f32, name="rg1", tag="s1", bufs=2)
                            nc.vector.reciprocal_approx_fast(rl1, sl1)
                            nc.vector.reciprocal_approx_fast(rg1, sg1)
                            gl1 = pc.tile([1, qcw], f32, name="gl1", tag="s1", bufs=2)
                            gg1 = pc.tile([1, qcw], f32, name="gg1", tag="s1", bufs=2)
                            nc.sync.dma_start(gl1, gt_d[0:1, h, qs])
                            nc.sync.dma_start(gg1, gt_d[1:2, h, qs])
                            cfl = pc.tile([1, qcw], bf16, name="cfl", tag="coefb", bufs=2)
                            cfg = pc.tile([1, qcw], bf16, name="cfg", tag="coefb", bufs=2)
                            nc.vector.tensor_mul(cfl, rl1, gl1)
                            nc.vector.tensor_mul(cfg, rg1, gg1)
                            bcl_s = pt.tile([64, qcw], bf16, name="bcl_s", tag="bcs")
                            bcg_s = pt.tile([64, qcw], bf16, name="bcg_s", tag="bcs")
                            nc.gpsimd.partition_broadcast(bcl_s, cfl)
                            nc.gpsimd.partition_broadcast(bcg_s, cfg)
                            t1 = pt.tile([64, qcw], f32, name="t1", tag="t")
                            t2 = pt.tile([64, qcw], f32, name="t2", tag="t")
                            nc.vector.tensor_mul(t1, ctl_s[0:64, :], bcl_s)
                            nc.vector.tensor_mul(t2, ctg_s[0:64, :], bcg_s)
                        else:
                            coef2 = pc.tile([2, qcw], bf16, name="coef2", tag="coefb", bufs=2)
                            nc.vector.tensor_mul(coef2, rec2, gtt)
                            bcl = psc.tile([64, qcw], f32, name="bcl", tag="sc")
                            bcg = psc.tile([64, qcw], f32, name="bcg", tag="sc")
                            for hlf in range(qcw // 512):
                                hs512 = slice(hlf * 512, (hlf + 1) * 512)
                                nc.tensor.matmul(bcl[:, hs512], lhsT=sel_sb[0:2, 0, :],
                                                 rhs=coef2[:, hs512], start=True, stop=True)
                                nc.tensor.matmul(bcg[:, hs512], lhsT=sel_sb[0:2, 1, :],
                                                 rhs=coef2[:, hs512], start=True, stop=True)
                            t1 = pt.tile([64, qcw], f32, name="t1", tag="t")
                            t2 = pt.tile([64, qcw], f32, name="t2", tag="t")
                            nc.vector.tensor_mul(t1, ctl_s[0:64, :], bcl)
                            nc.vector.tensor_mul(t2, ctg_s[0:64, :], bcg)
                        o = po.tile([64, qcw], f32, name="o", tag="o")
                        nc.vector.tensor_add(o, t1, t2)
                        nc.sync.dma_start(out_d[h * 64:(h + 1) * 64, qs], o)
                        continue
                    # epilogue: sums (psum row 64) -> [2, qcw] at base partition 0,
                    # recip * gate, broadcast via K=2 selector matmuls, combine.
                    stage = pc.tile([65, 2, qcw], f32, name="stage", tag="stage",
                                    bufs=1 if tune else 2)
                    if tune:
                        nc.vector.tensor_copy(stage[64:65, 0, :], ctl[64:65, :])
                        nc.vector.tensor_copy(stage[64:65, 1, :], ctg[64:65, :])
                    else:
                        nc.scalar.activation(stage[64:65, 0, :], ctl[64:65, :], AF.Copy)
                        nc.scalar.activation(stage[64:65, 1, :], ctg[64:65, :], AF.Copy)
                    sums2 = pc.tile([2, qcw], f32, name="sums2", tag="sums", bufs=2)
                    nc.sync.dma_start(sums2, stage[64:65, :, :])
                    rec2 = pc.tile([2, qcw], f32, name="rec2", tag="sums", bufs=2)
                    nc.vector.reciprocal_approx_fast(rec2, sums2)
                    gtt = pc.tile([2, qcw], f32, name="gtt", tag="gtt", bufs=2)
                    nc.sync.dma_start(gtt, gt_d[:, h, qs])
                    coef2 = pc.tile([2, qcw], bf16, name="coef2", tag="coefb", bufs=2)
                    nc.vector.tensor_mul(coef2, rec2, gtt)
                    bcl = psc.tile([64, qcw], f32, name="bcl", tag="sc")
                    bcg = psc.tile([64, qcw], f32, name="bcg", tag="sc")
                    for hlf in range(qcw // 512):
                        hs512 = slice(hlf * 512, (hlf + 1) * 512)
                        nc.tensor.matmul(
                            bcl[:, hs512],
                            lhsT=sel_sb[0:2, 0, :],
                            rhs=coef2[:, hs512],
                            start=True, stop=True)
                        nc.tensor.matmul(
                            bcg[:, hs512],
                            lhsT=sel_sb[0:2, 1, :],
                            rhs=coef2[:, hs512],
                            start=True, stop=True)
                    bcl_s = pt.tile([64, qcw], f32, name="bcl_s", tag="bc")
                    bcg_s = pt.tile([64, qcw], f32, name="bcg_s", tag="bc")
                    nc.scalar.activation(bcl_s, bcl, AF.Copy)
                    nc.scalar.activation(bcg_s, bcg, AF.Copy)
                    t1 = pt.tile([64, qcw], f32, name="t1", tag="t")
                    t2 = pt.tile([64, qcw], f32, name="t2", tag="t")
                    nc.vector.tensor_mul(t1, ctl[0:64, :], bcl_s)
                    nc.vector.tensor_mul(t2, ctg[0:64, :], bcg_s)
                    o = po.tile([64, qcw], f32, name="o", tag="o")
                    nc.vector.tensor_add(o, t1, t2)
                    nc.sync.dma_start(out_d[h * 64:(h + 1) * 64, qs], o)
            att_ctx.close()

    nc.compile()
    return nc


def _get(use_em: bool, has_b: bool):
    key = (use_em, has_b)
    if key not in _BUILT:
        _BUILT[key] = _build(use_em, has_b=has_b)
    return _BUILT[key]


def _prep_core(c, hs, am, lm, go, Wq, bq, Wk, bk, Wv, bv, use_em):
    bf = ml_dtypes.bfloat16
    b, hg = c // 4, c % 4
    h0 = hg * HPC
    sl = slice(h0 * DH, (h0 + HPC) * DH)
    m = {
        "xt": np.ascontiguousarray(hs[b].T).astype(bf),
        "wq": np.ascontiguousarray((Wq[sl, :] / 8.0).T).astype(bf),
        "wk": np.ascontiguousarray(Wk[sl, :].T).astype(bf),
        "wv": np.ascontiguousarray(Wv[sl, :].T).astype(bf),
        "bqk": np.stack([bq[sl] / 8.0, bk[sl]]).astype(np.float32),
        "bv": bv[sl].reshape(1, 256).astype(bf),
        "msk": np.ascontiguousarray(
            lm[b, 0].astype(np.float32).T).reshape(KT, 128, S).astype(bf),
        "gt": np.stack([
            np.stack([go[b, h0 + j, :, 0] for j in range(HPC)]),
            np.stack([1.0 - go[b, h0 + j, :, 0] for j in range(HPC)]),
        ]).astype(np.float32),
        "sel": np.broadcast_to(
            np.eye(4, dtype=np.float32)[:, :, None], (4, 4, 64)).astype(bf),
    }
    if use_em:
        m["em"] = np.exp(am[b, 0, 0]).astype(np.float32).reshape(KT, 128)
    return m


def make_in_maps(inputs):
    hs = np.asarray(inputs["hidden_states"], np.float32)
    am = np.asarray(inputs["attention_mask"], np.float32)
    lm = np.asarray(inputs["local_attention_mask"])
    go = np.asarray(inputs["gate_outputs"], np.float32)
    Wq = np.asarray(inputs["Wq"], np.float32)
    bq = np.asarray(inputs["bq"], np.float32)
    Wk = np.asarray(inputs["Wk"], np.float32)
    bk = np.asarray(inputs["bk"], np.float32)
    Wv = np.asarray(inputs["Wv"], np.float32)
    bv = np.asarray(inputs["bv"], np.float32)
    use_em = bool(np.any(am != 0.0))
    has_b = bool(np.any(bq != 0.0) or np.any(bk != 0.0) or np.any(bv != 0.0))
    maps = [
        _prep_core(c, hs, am, lm, go, Wq, bq, Wk, bk, Wv, bv, use_em)
        for c in range(NCORES)
    ]
    return maps, (use_em, has_b)


def assemble(results):
    out = np.empty((B, S, H), np.float32)
    for c in range(NCORES):
        b, hg = c // 4, c % 4
        sl = slice(hg * HPC * DH, (hg + 1) * HPC * DH)
        out[b, :, sl] = np.asarray(results[c]["out"]).T
    return out


def kernel(**inputs):
    from concourse import bass_utils

    maps, (use_em, has_b) = make_in_maps(inputs)
    nc = _get(use_em, has_b)
    res = bass_utils.run_bass_kernel_spmd(nc, maps, core_ids=list(range(NCORES)))
    return assemble(res.results)

